# revision 7
# baseline (speedup 1.0000x reference)
"""MoE routing layer on 8 Trainium2 NeuronCores (data-parallel over batch).

Per core (4 samples):
  routing MLP -> cosine sim vs embeddings -> softmax weights wf[4,10]
  w_eff[b] = sum_n wf[b,n] * conv_w[n]  (conv is linear in weights ->
  10x fewer conv FLOPs than materializing all expert convs)
  out[b] = conv2d(x[b], w_eff[b]) + b_eff[b]

Conv path runs in bf16 (x, w_eff) accumulating fp32 in PSUM. The conv is
9 shifted matmuls over the flat 58-wide grid; the PE array is quad-tiled
(2 samples on row halves x 2 chunk parities on column halves) so all four
64x64 quadrants stream concurrently. w_eff is a 10-expert weighted sum
built from independent tensor_scalar multiplies (4x DVE mode) plus a
tensor_tensor add tree, split across the vector and gpsimd engines. All
DMA goes through the sync engine's hardware queue (a gpsimd-dispatched
load would stall gpsimd behind an expensive DGE drain).
"""
import sys

sys.path.insert(0, "/opt/trn_rl_repo")

import numpy as np
import ml_dtypes

import concourse.bass as bass
import concourse.mybir as mybir
from concourse.tile import TileContext

F32 = mybir.dt.float32
BF16 = mybir.dt.bfloat16
AF = mybir.ActivationFunctionType
ALU = mybir.AluOpType
AX = mybir.AxisListType

NCORES = 8
BLOC = 4           # samples per core
CIN = 64
COUT = 64
H = W = 58
HW = H * W         # 3364
OH = OW = 56
NB = 10            # experts
EDIM = 64
RSIZE = 512
HID = 128
NTAP = 9
GR = 7             # output rows per chunk
NGRP = 4           # 4 groups x (even chunk + odd chunk) x 7 rows = 56
NFREE = GR * W     # 406 <= 512 (one PSUM bank)
TAP_OFF = [dy * W + dx for dy in range(3) for dx in range(3)]
NWARM_PRE = 10     # PE warmup matmuls (HAM clock-gate) after routing
NWARM_POST = 16    # bridge the PE gap from routing to first conv matmul
NB_A = 5           # experts 0:5 land first (vector), 5:10 second (gpsimd)

# blkb column layout (128 partitions, bf16): routing operands
BB_RVT = 0                  # [128, 4c, 4b]
BB_W1 = BB_RVT + 16         # [128, 4c, 128m]
BB_W2 = BB_W1 + 512         # [128, 64]
BB_D = BB_W2 + 64           # 592

# blk2 column layout (128 partitions, fp32)
B2_EMB = 0                  # [10, 64]
B2_CB = B2_EMB + 64         # [10, 64]
B2_ID = B2_CB + 64          # [16, 16]
B2_SEL = B2_ID + 16         # [4, 2j, 128m]
B2_B1 = B2_SEL + 256        # [128, 1]
B2_B2 = B2_B1 + 1           # [64, 1]
B2_D = B2_B2 + 1            # 402


def fix_sync_waits(nc, cap=2):
    """This walrus build allows at most `cap` sem waits per instruction.
    Splice same-engine NoOps carrying the excess waits right before any
    over-subscribed instruction (waits happen earlier => same semantics)."""
    uid = [0]
    for f in nc.m.functions:
        for blk in f.blocks:
            insts = blk.instructions  # live list
            i = 0
            while i < len(insts):
                inst = insts[i]
                si = inst.sync_info
                waits = list(si.on_wait) if si and si.on_wait else []
                icap = 1
                if len(waits) <= icap:
                    i += 1
                    continue
                keep, excess = waits[-icap:], waits[:-icap]
                for k in range(0, len(excess), icap):
                    nop = mybir.InstNoOp(
                        name=f"{inst.name}-wsplit{uid[0]}", ins=[], outs=[]
                    )
                    uid[0] += 1
                    nop.engine = inst.engine
                    nop.sync_info = mybir.SyncInfo(
                        on_wait=excess[k : k + icap], on_update=[]
                    )
                    nc.register_instruction(nop, overwrite=True)
                    insts.insert(i, nop)
                    i += 1
                inst.sync_info = mybir.SyncInfo(
                    on_wait=keep,
                    on_update=list(si.on_update) if si and si.on_update else [],
                )
                i += 1


def build():
    nc = bass.Bass(num_swdge_queues=4)
    blkb = nc.dram_tensor("blkb", [128, BB_D], BF16, kind="ExternalInput")
    blk2 = nc.dram_tensor("blk2", [128, B2_D], F32, kind="ExternalInput")
    cwp = nc.dram_tensor("cwp", [CIN, NB, NTAP, COUT], BF16, kind="ExternalInput")
    x = nc.dram_tensor("x", [BLOC, CIN, HW], BF16, kind="ExternalInput")
    out = nc.dram_tensor("out", [BLOC, COUT, OH, OW], F32, kind="ExternalOutput")

    with TileContext(nc) as tc:
        with (
            tc.tile_pool(name="consts", bufs=1) as consts,
            tc.tile_pool(name="work", bufs=2) as work,
            tc.tile_pool(name="stage", bufs=3) as stage,
            tc.tile_pool(name="ps", bufs=2, space="PSUM") as pspool,
            tc.tile_pool(name="psconv", bufs=2, space="PSUM") as psconv,
            tc.tile_pool(name="pswarm", bufs=1, space="PSUM") as pswarm,
        ):
            # ---------- DMA in (all on sync hw queue, critical first) ----------
            bbt = consts.tile([128, BB_D], BF16, tag="bbt")
            nc.sync.dma_start(out=bbt[:], in_=blkb[:])
            b2t = consts.tile([128, B2_D], F32, tag="b2t")
            nc.sync.dma_start(out=b2t[:], in_=blk2[:])
            cwp2 = consts.tile([128, NB, NTAP, COUT], BF16, tag="cwp2")
            nc.sync.dma_start(out=cwp2[0:64, 0:NB_A], in_=cwp[:, 0:NB_A])
            nc.sync.dma_start(out=cwp2[64:128, 0:NB_A], in_=cwp[:, 0:NB_A])
            nc.sync.dma_start(out=cwp2[0:64, NB_A:NB], in_=cwp[:, NB_A:NB])
            nc.sync.dma_start(out=cwp2[64:128, NB_A:NB], in_=cwp[:, NB_A:NB])

            xt = []
            for j in range(2):
                t = consts.tile([128, HW + 4], BF16, tag=f"xt{j}")
                nc.vector.memset(t[:, HW : HW + 4], 0.0)
                xt.append(t)
            for b in range(BLOC):
                j, half = divmod(b, 2)
                nc.sync.dma_start(
                    out=xt[j][64 * half : 64 * half + 64, 0:HW], in_=x[b]
                )

            ones64 = consts.tile([EDIM, 1], F32, tag="ones64")
            nc.vector.memset(ones64[:], 1.0)

            # views into the packed blocks
            rvT = bbt[:, BB_RVT : BB_RVT + 16].rearrange("p (c b) -> p c b", c=4)
            w1sb = bbt[:, BB_W1 : BB_W1 + 512].rearrange("p (c m) -> p c m", c=4)
            w2sb = bbt[:, BB_W2 : BB_W2 + 64]
            embsb = b2t[0:NB, B2_EMB : B2_EMB + 64]
            cbsb = b2t[0:NB, B2_CB : B2_CB + 64]
            ident = b2t[0:16, B2_ID : B2_ID + 16]
            selsb = b2t[0:4, B2_SEL : B2_SEL + 256].rearrange(
                "p (j m) -> p j m", j=2
            )
            b1sb = b2t[:, B2_B1 : B2_B1 + 1]
            b2sb = b2t[0:EDIM, B2_B2 : B2_B2 + 1]

            # ---------- routing MLP (bf16 matmuls; rv pre-transposed) ----------
            h1 = pspool.tile([HID, BLOC], F32, tag="small")
            for c in range(4):
                nc.tensor.matmul(
                    h1[:], w1sb[:, c, :], rvT[:, c, :], start=(c == 0), stop=(c == 3)
                )
            h1r = work.tile([HID, BLOC], BF16, tag="h1r")
            nc.scalar.activation(
                out=h1r[:], in_=h1[:], func=AF.Relu, bias=b1sb, scale=1.0
            )
            rps = pspool.tile([EDIM, BLOC], F32, tag="small")
            nc.tensor.matmul(rps[:], w2sb, h1r[:], start=True, stop=True)
            rsb = work.tile([EDIM, BLOC], F32, tag="rsb")
            nc.scalar.activation(
                out=rsb[:], in_=rps[:], func=AF.Identity, bias=b2sb, scale=1.0
            )

            # ---------- cosine similarity ----------
            rsq = work.tile([EDIM, BLOC], F32, tag="rsq")
            nc.vector.tensor_mul(rsq[:], rsb[:], rsb[:])
            nsq = pspool.tile([BLOC, 1], F32, tag="small")
            nc.tensor.matmul(nsq[:], rsq[:], ones64[:], start=True, stop=True)
            rln = work.tile([BLOC, 1], F32, tag="rln")
            nc.scalar.activation(out=rln[:], in_=nsq[:], func=AF.Ln)
            rinv = work.tile([BLOC, 1], F32, tag="rinv")
            nc.scalar.activation(out=rinv[:], in_=rln[:], func=AF.Exp, scale=-0.5)

            esq = work.tile([NB, EDIM], F32, tag="esq")
            nc.vector.tensor_mul(esq[:], embsb, embsb)
            ensq = work.tile([NB, 1], F32, tag="ensq")
            nc.vector.tensor_reduce(ensq[:], esq[:], axis=AX.X, op=ALU.add)
            eln = work.tile([NB, 1], F32, tag="eln")
            nc.scalar.activation(out=eln[:], in_=ensq[:], func=AF.Ln)
            einv = work.tile([NB, 1], F32, tag="einv")
            nc.scalar.activation(out=einv[:], in_=eln[:], func=AF.Exp, scale=-0.5)
            embn = work.tile([NB, EDIM], F32, tag="embn")
            nc.vector.tensor_scalar_mul(out=embn[:], in0=embsb, scalar1=einv[:])
            embnT_ps = pspool.tile([EDIM, NB], F32, tag="small")
            nc.tensor.transpose(embnT_ps[:], embn[:], ident[0:NB, 0:NB])
            embnT = work.tile([EDIM, NB], F32, tag="embnT")
            nc.scalar.copy(out=embnT[:], in_=embnT_ps[:])

            simps = pspool.tile([BLOC, NB], F32, tag="small")
            nc.tensor.matmul(simps[:], rsb[:], embnT[:], start=True, stop=True)
            sim = work.tile([BLOC, NB], F32, tag="sim")
            nc.vector.tensor_scalar_mul(out=sim[:], in0=simps[:], scalar1=rinv[:])

            # ---------- softmax ----------
            mx = work.tile([BLOC, 1], F32, tag="mx")
            nc.vector.tensor_reduce(mx[:], sim[:], axis=AX.X, op=ALU.max)
            negmx = work.tile([BLOC, 1], F32, tag="negmx")
            nc.vector.tensor_scalar_mul(out=negmx[:], in0=mx[:], scalar1=-1.0)
            ex = work.tile([BLOC, NB], F32, tag="ex")
            s = work.tile([BLOC, 1], F32, tag="s")
            nc.scalar.activation(
                out=ex[:], in_=sim[:], func=AF.Exp, bias=negmx[:], scale=1.0,
                accum_out=s[:],
            )
            sinv = work.tile([BLOC, 1], F32, tag="sinv")
            nc.vector.reciprocal(sinv[:], s[:])
            wf = work.tile([BLOC, NB], F32, tag="wf")
            nc.vector.tensor_scalar_mul(out=wf[:], in0=ex[:], scalar1=sinv[:])

            # ---------- PE warmup: keep HAM clock ungated until conv ----------
            wl = bbt[:, BB_W1 : BB_W1 + 128]
            wr = bbt[:, BB_W1 : BB_W1 + 512]
            warm_ps = pswarm.tile([128, 512], F32, tag="warm")
            for _ in range(NWARM_PRE):
                nc.tensor.matmul(warm_ps[:], wl, wr, start=True, stop=True)

            # ---------- effective conv bias (both partition halves) ----------
            wfT_ps = pspool.tile([NB, BLOC], F32, tag="small")
            nc.tensor.transpose(wfT_ps[:], wf[:], ident[0:BLOC, 0:BLOC])
            wfT = work.tile([NB, BLOC], F32, tag="wfT")
            nc.scalar.copy(out=wfT[:], in_=wfT_ps[:])
            beff_ps = pspool.tile([128, BLOC], F32, tag="small")
            nc.tensor.matmul(
                beff_ps[0:64], cbsb, wfT[:], start=True, stop=True,
                tile_position=(0, 0),
            )
            nc.tensor.matmul(
                beff_ps[64:128], cbsb, wfT[:], start=True, stop=True,
                tile_position=(0, 64), skip_group_check=True,
            )
            beff2 = work.tile([128, BLOC], F32, tag="beff2")
            nc.scalar.copy(out=beff2[:], in_=beff_ps[:])

            # ---------- per-pair expert weight broadcast ----------
            wfbcs = []
            for j in range(2):
                wfbc_ps = pspool.tile([128, NB], F32, tag="small")
                nc.tensor.matmul(
                    wfbc_ps[:], selsb[:, j, :], wf[:], start=True, stop=True
                )
                wfbc = work.tile([128, NB], F32, tag=f"wfbc{j}")
                nc.scalar.copy(out=wfbc[:], in_=wfbc_ps[:])
                wfbcs.append(wfbc)

            for _ in range(NWARM_POST):
                nc.tensor.matmul(warm_ps[:], wl, wr, start=True, stop=True)
            warm_sink = work.tile([1, 1], F32, tag="warm_sink")
            nc.scalar.copy(out=warm_sink[:], in_=warm_ps[0:1, 0:1])

            # ---------- w_eff: tensor_scalar multiplies + add tree ----------
            # weff[0:64]  = weights for sample 2j   (PE row tile 0)
            # weff[64:128] = weights for sample 2j+1 (PE row tile 64)
            # experts 0:4 + partial tree on vector (cwp first half arrives
            # early), experts 5:9 on gpsimd, final combine on vector.
            FD = NTAP * COUT  # 576
            cv = cwp2[:].rearrange("p n t c -> p n (t c)")
            weffs = []
            for j in range(2):
                wfbc = wfbcs[j]
                tt = [
                    work.tile([128, FD], BF16, tag=f"t{n}_{j}", name=f"t{n}_{j}")
                    for n in range(NB)
                ]
                for n in range(5):
                    nc.vector.tensor_scalar_mul(
                        out=tt[n][:], in0=cv[:, n, :], scalar1=wfbc[:, n : n + 1]
                    )
                for n in range(5, NB):
                    nc.gpsimd.tensor_scalar_mul(
                        out=tt[n][:], in0=cv[:, n, :], scalar1=wfbc[:, n : n + 1]
                    )
                u0 = work.tile([128, FD], BF16, tag=f"u0_{j}", name=f"u0_{j}")
                u1 = work.tile([128, FD], BF16, tag=f"u1_{j}", name=f"u1_{j}")
                v0 = work.tile([128, FD], BF16, tag=f"v0_{j}", name=f"v0_{j}")
                w0 = work.tile([128, FD], BF16, tag=f"w0_{j}", name=f"w0_{j}")
                nc.vector.tensor_add(u0[:], tt[0][:], tt[1][:])
                nc.vector.tensor_add(u1[:], tt[2][:], tt[3][:])
                nc.vector.tensor_add(v0[:], u0[:], u1[:])
                nc.vector.tensor_add(w0[:], v0[:], tt[4][:])
                u2 = work.tile([128, FD], BF16, tag=f"u2_{j}", name=f"u2_{j}")
                u3 = work.tile([128, FD], BF16, tag=f"u3_{j}", name=f"u3_{j}")
                w1_ = work.tile([128, FD], BF16, tag=f"w1_{j}", name=f"w1_{j}")
                nc.gpsimd.tensor_add(u2[:], tt[5][:], tt[6][:])
                nc.gpsimd.tensor_add(u3[:], tt[7][:], tt[8][:])
                nc.gpsimd.tensor_add(w1_[:], u2[:], u3[:])
                wsum = work.tile([128, FD], BF16, tag=f"ws_{j}", name=f"ws_{j}")
                nc.gpsimd.tensor_add(wsum[:], w1_[:], tt[9][:])
                weff = work.tile(
                    [128, NTAP, COUT], BF16, tag=f"weff{j}", name=f"weff{j}"
                )
                nc.vector.tensor_add(
                    weff[:].rearrange("p t c -> p (t c)"), w0[:], wsum[:]
                )
                weffs.append(weff)

            # ---------- conv: quad-tiled 9-tap shifted matmuls ----------
            stgs = [None, None]
            for j in range(2):
                weff = weffs[j]
                for g in range(NGRP):
                    h_e = 2 * g * GR       # even chunk start row
                    h_o = h_e + GR         # odd chunk start row
                    psA = psconv.tile([128, NFREE], F32, tag="psA")
                    psB = psconv.tile([128, NFREE], F32, tag="psB")
                    for t in range(NTAP):
                        oe = h_e * W + TAP_OFF[t]
                        oo = h_o * W + TAP_OFF[t]
                        st_, sp = (t == 0), (t == NTAP - 1)
                        nc.tensor.matmul(
                            psA[0:64], weff[0:64, t, :], xt[j][0:64, oe : oe + NFREE],
                            start=st_, stop=sp, tile_position=(0, 0),
                            skip_group_check=True,
                        )
                        nc.tensor.matmul(
                            psA[64:128], weff[0:64, t, :], xt[j][0:64, oo : oo + NFREE],
                            start=st_, stop=sp, tile_position=(0, 64),
                            skip_group_check=True,
                        )
                        nc.tensor.matmul(
                            psB[0:64], weff[64:128, t, :], xt[j][64:128, oe : oe + NFREE],
                            start=st_, stop=sp, tile_position=(64, 0),
                            skip_group_check=True,
                        )
                        nc.tensor.matmul(
                            psB[64:128], weff[64:128, t, :], xt[j][64:128, oo : oo + NFREE],
                            start=st_, stop=sp, tile_position=(64, 64),
                            skip_group_check=True,
                        )
                    # evacuate both parities in one op per sample (scalar),
                    # +bias, trim 58->56 cols
                    gi = g % 2
                    for half, ps in ((0, psA), (1, psB)):
                        b = 2 * j + half
                        if gi == 0:
                            stgs[half] = stage.tile(
                                [128, 2, GR, OW], F32,
                                tag=f"stg{half}", name=f"stg{half}",
                            )
                        st = stgs[half]
                        pv = ps[:].rearrange("p (r w) -> p r w", w=W)[:, :, 0:OW]
                        nc.scalar.activation(
                            out=st[:, gi], in_=pv, func=AF.Identity,
                            bias=beff2[:, b : b + 1], scale=1.0,
                        )
                        # after 2 groups: store both row-parities of the window
                        if gi == 1:
                            dv = out[b].rearrange(
                                "c (G g2 r) w -> g2 c G r w", G=NGRP, g2=2
                            )
                            nc.sync.dma_start(
                                out=dv[0, :, g - 1 : g + 1], in_=st[0:64]
                            )
                            nc.sync.dma_start(
                                out=dv[1, :, g - 1 : g + 1], in_=st[64:128]
                            )

    fix_sync_waits(nc)
    return nc


_NC = None


def _get_nc():
    global _NC
    if _NC is None:
        _NC = build()
    return _NC


def make_in_maps(inputs):
    bf16 = ml_dtypes.bfloat16
    x = np.asarray(inputs["x"], dtype=np.float32).reshape(32, CIN, HW)
    rvec = np.asarray(inputs["routing_vector"], dtype=np.float32)
    W1 = np.asarray(inputs["W1"], dtype=np.float32)
    b1 = np.asarray(inputs["b1"], dtype=np.float32)
    W2 = np.asarray(inputs["W2"], dtype=np.float32)
    b2 = np.asarray(inputs["b2"], dtype=np.float32)
    emb = np.asarray(inputs["emb"], dtype=np.float32)
    conv_w = np.asarray(inputs["conv_w"], dtype=np.float32)
    conv_b = np.asarray(inputs["conv_b"], dtype=np.float32)

    # conv_w[n, co, ci, ky, kx] -> cwp[ci, n, (ky kx), co], bf16
    cwpa = np.ascontiguousarray(
        conv_w.transpose(2, 0, 3, 4, 1).reshape(CIN, NB, NTAP, COUT)
    ).astype(bf16)

    # blkb (bf16): per-core rvT + shared routing weights
    w1blk = W1.reshape(4, 128, HID).transpose(1, 0, 2).reshape(128, 512)
    blkb_shared = np.zeros((128, BB_D), np.float32)
    blkb_shared[:, BB_W1 : BB_W1 + 512] = w1blk
    blkb_shared[:, BB_W2 : BB_W2 + 64] = W2

    # blk2 (fp32): emb, conv bias, identity, sel masks, b1, b2
    selm = np.zeros((4, 2, 128), np.float32)
    for j in range(2):
        selm[2 * j, j, 0:64] = 1.0
        selm[2 * j + 1, j, 64:128] = 1.0
    blk2a = np.zeros((128, B2_D), np.float32)
    blk2a[0:NB, B2_EMB : B2_EMB + 64] = emb
    blk2a[0:NB, B2_CB : B2_CB + 64] = conv_b
    blk2a[0:16, B2_ID : B2_ID + 16] = np.eye(16, dtype=np.float32)
    blk2a[0:4, B2_SEL : B2_SEL + 256] = selm.reshape(4, 256)
    blk2a[:, B2_B1] = b1
    blk2a[0:EDIM, B2_B2] = b2

    xb = x.astype(bf16)
    in_maps = []
    for c in range(NCORES):
        blkba = blkb_shared.copy()
        rvc = rvec[BLOC * c : BLOC * (c + 1)]          # [4, 512]
        # rvT[p, c, b] = rv[b, 128c + p]
        rvt = rvc.T.reshape(4, 128, BLOC).transpose(1, 0, 2).reshape(128, 16)
        blkba[:, BB_RVT : BB_RVT + 16] = rvt
        in_maps.append(
            {
                "blkb": blkba.astype(bf16),
                "blk2": blk2a,
                "cwp": cwpa,
                "x": np.ascontiguousarray(xb[BLOC * c : BLOC * (c + 1)]),
            }
        )
    return in_maps


def kernel(**inputs):
    from concourse.bass_utils import run_bass_kernel_spmd

    nc = _get_nc()
    in_maps = make_in_maps(inputs)
    res = run_bass_kernel_spmd(nc, in_maps, core_ids=list(range(NCORES)))
    return np.concatenate([r["out"] for r in res.results], axis=0)


# revision 8
# speedup vs baseline: 2.9329x; 2.9329x over previous
"""MoE routing layer on 8 Trainium2 NeuronCores (data-parallel over batch).

Per core (4 samples):
  routing MLP -> cosine sim vs embeddings -> softmax weights wf[4,10]
  w_eff[b] = sum_n wf[b,n] * conv_w[n]  (conv is linear in weights ->
  10x fewer conv FLOPs than materializing all expert convs)
  out[b] = conv2d(x[b], w_eff[b]) + b_eff[b]

Conv path runs in bf16 (x, w_eff) accumulating fp32 in PSUM. The conv is
9 shifted matmuls over the flat 58-wide grid; the PE array is quad-tiled
(2 samples on row halves x 2 chunk parities on column halves) so all four
64x64 quadrants stream concurrently.

w_eff is ALSO built on the PE: conv weights are host-packed as expert
pairs on partition halves (cwp[(n2 ci), g, tap*cout]), and for each
expert-pair g a [128,64] stationary operand holding two stacked scaled
identities (wf[b,2g]*I ; wf[b,2g+1]*I) contracts against it, accumulating
the 10-expert weighted sum directly in PSUM. The DVE only builds the
small scaled identities — the [128,576]-sized elementwise work that made
vector/gpsimd the bottleneck is gone.
"""
import sys

sys.path.insert(0, "/opt/trn_rl_repo")

import numpy as np
import ml_dtypes

import concourse.bass as bass
import concourse.mybir as mybir
from concourse.tile import TileContext

F32 = mybir.dt.float32
BF16 = mybir.dt.bfloat16
AF = mybir.ActivationFunctionType
ALU = mybir.AluOpType
AX = mybir.AxisListType

NCORES = 8
BLOC = 4           # samples per core
CIN = 64
COUT = 64
H = W = 58
HW = H * W         # 3364
OH = OW = 56
NB = 10            # experts
NG = 5             # expert pairs
EDIM = 64
RSIZE = 512
HID = 128
NTAP = 9
FD = NTAP * COUT   # 576
FDH = FD // 2      # 288 (PSUM bank-sized half)
GR = 7             # output rows per chunk
NGRP = 4           # 4 groups x (even chunk + odd chunk) x 7 rows = 56
NFREE = GR * W     # 406 <= 512 (one PSUM bank)
TAP_OFF = [dy * W + dx for dy in range(3) for dx in range(3)]
NWARM_PRE = 10     # PE warmup matmuls (HAM clock-gate) after routing
NWARM_POST = 16    # bridge the PE gap from routing to the w_eff matmuls

# blkb column layout (128 partitions, bf16): routing weights + identities
BB_RVT = 0                  # [128, 4c, 4b]
BB_W1 = BB_RVT + 16         # [128, 4c, 128m]
BB_W2 = BB_W1 + 512         # [128, 64]
BB_IP = BB_W2 + 64          # [128, 64] two stacked 64x64 identities
BB_D = BB_IP + 64           # 656

# blk2 column layout (128 partitions, fp32)
B2_EMB = 0                  # [10, 64]
B2_CB = B2_EMB + 64         # [10, 64]
B2_ID = B2_CB + 64          # [16, 16]
B2_E4 = B2_ID + 16          # [4, 20]  E4[b',(b g)] = delta(b,b')
B2_ME = B2_E4 + 20          # [10, 20] mask_even[n,(b g)] = delta(n,2g)
B2_MO = B2_ME + 20          # [10, 20] mask_odd[n,(b g)]  = delta(n,2g+1)
B2_B1 = B2_MO + 20          # [128, 1]
B2_B2 = B2_B1 + 1           # [64, 1]
B2_D = B2_B2 + 1            # 206


def fix_sync_waits(nc, cap=2):
    """This walrus build allows at most `cap` sem waits per instruction.
    Splice same-engine NoOps carrying the excess waits right before any
    over-subscribed instruction (waits happen earlier => same semantics)."""
    uid = [0]
    for f in nc.m.functions:
        for blk in f.blocks:
            insts = blk.instructions  # live list
            i = 0
            while i < len(insts):
                inst = insts[i]
                si = inst.sync_info
                waits = list(si.on_wait) if si and si.on_wait else []
                icap = 1
                if len(waits) <= icap:
                    i += 1
                    continue
                keep, excess = waits[-icap:], waits[:-icap]
                for k in range(0, len(excess), icap):
                    nop = mybir.InstNoOp(
                        name=f"{inst.name}-wsplit{uid[0]}", ins=[], outs=[]
                    )
                    uid[0] += 1
                    nop.engine = inst.engine
                    nop.sync_info = mybir.SyncInfo(
                        on_wait=excess[k : k + icap], on_update=[]
                    )
                    nc.register_instruction(nop, overwrite=True)
                    insts.insert(i, nop)
                    i += 1
                inst.sync_info = mybir.SyncInfo(
                    on_wait=keep,
                    on_update=list(si.on_update) if si and si.on_update else [],
                )
                i += 1


def build():
    nc = bass.Bass(num_swdge_queues=4)
    blkb = nc.dram_tensor("blkb", [128, BB_D], BF16, kind="ExternalInput")
    blk2 = nc.dram_tensor("blk2", [128, B2_D], F32, kind="ExternalInput")
    cwp = nc.dram_tensor("cwp", [128, NG, FD], BF16, kind="ExternalInput")
    x = nc.dram_tensor("x", [BLOC, CIN, HW], BF16, kind="ExternalInput")
    out = nc.dram_tensor("out", [BLOC, COUT, OH, OW], F32, kind="ExternalOutput")

    with TileContext(nc) as tc:
        with (
            tc.tile_pool(name="consts", bufs=1) as consts,
            tc.tile_pool(name="work", bufs=2) as work,
            tc.tile_pool(name="stage", bufs=3) as stage,
            tc.tile_pool(name="ps", bufs=2, space="PSUM") as pspool,
            tc.tile_pool(name="psconv", bufs=2, space="PSUM") as psconv,
            tc.tile_pool(name="pswarm", bufs=1, space="PSUM") as pswarm,
        ):
            # ---------- DMA in (all on sync hw queue, critical first) ----------
            bbt = consts.tile([128, BB_D], BF16, tag="bbt")
            nc.sync.dma_start(out=bbt[:], in_=blkb[:])
            b2t = consts.tile([128, B2_D], F32, tag="b2t")
            nc.sync.dma_start(out=b2t[:], in_=blk2[:])
            cwt = consts.tile([128, NG, FD], BF16, tag="cwt")
            nc.sync.dma_start(out=cwt[:, 0:3], in_=cwp[:, 0:3])
            nc.sync.dma_start(out=cwt[:, 3:NG], in_=cwp[:, 3:NG])

            xt = []
            for j in range(2):
                t = consts.tile([128, HW + 4], BF16, tag=f"xt{j}")
                nc.vector.memset(t[:, HW : HW + 4], 0.0)
                xt.append(t)
            for b in range(BLOC):
                j, half = divmod(b, 2)
                nc.sync.dma_start(
                    out=xt[j][64 * half : 64 * half + 64, 0:HW], in_=x[b]
                )

            ones64 = consts.tile([EDIM, 1], F32, tag="ones64")
            nc.vector.memset(ones64[:], 1.0)
            ones10_64 = consts.tile([NB, 64], F32, tag="ones10_64")
            nc.vector.memset(ones10_64[:], 1.0)

            # views into the packed blocks
            rvT = bbt[:, BB_RVT : BB_RVT + 16].rearrange("p (c b) -> p c b", c=4)
            w1sb = bbt[:, BB_W1 : BB_W1 + 512].rearrange("p (c m) -> p c m", c=4)
            w2sb = bbt[:, BB_W2 : BB_W2 + 64]
            identPair = bbt[:, BB_IP : BB_IP + 64]
            embsb = b2t[0:NB, B2_EMB : B2_EMB + 64]
            cbsb = b2t[0:NB, B2_CB : B2_CB + 64]
            ident = b2t[0:16, B2_ID : B2_ID + 16]
            e4sb = b2t[0:4, B2_E4 : B2_E4 + 20]
            maskE = b2t[0:NB, B2_ME : B2_ME + 20]
            maskO = b2t[0:NB, B2_MO : B2_MO + 20]
            b1sb = b2t[:, B2_B1 : B2_B1 + 1]
            b2sb = b2t[0:EDIM, B2_B2 : B2_B2 + 1]

            # ---------- routing MLP (bf16 matmuls; rv pre-transposed) ----------
            h1 = pspool.tile([HID, BLOC], F32, tag="small")
            for c in range(4):
                nc.tensor.matmul(
                    h1[:], w1sb[:, c, :], rvT[:, c, :], start=(c == 0), stop=(c == 3)
                )
            h1r = work.tile([HID, BLOC], BF16, tag="h1r")
            nc.scalar.activation(
                out=h1r[:], in_=h1[:], func=AF.Relu, bias=b1sb, scale=1.0
            )
            rps = pspool.tile([EDIM, BLOC], F32, tag="small")
            nc.tensor.matmul(rps[:], w2sb, h1r[:], start=True, stop=True)
            rsb = work.tile([EDIM, BLOC], F32, tag="rsb")
            nc.scalar.activation(
                out=rsb[:], in_=rps[:], func=AF.Identity, bias=b2sb, scale=1.0
            )

            # ---------- cosine similarity ----------
            rsq = work.tile([EDIM, BLOC], F32, tag="rsq")
            nc.vector.tensor_mul(rsq[:], rsb[:], rsb[:])
            nsq = pspool.tile([BLOC, 1], F32, tag="small")
            nc.tensor.matmul(nsq[:], rsq[:], ones64[:], start=True, stop=True)
            rln = work.tile([BLOC, 1], F32, tag="rln")
            nc.scalar.activation(out=rln[:], in_=nsq[:], func=AF.Ln)
            rinv = work.tile([BLOC, 1], F32, tag="rinv")
            nc.scalar.activation(out=rinv[:], in_=rln[:], func=AF.Exp, scale=-0.5)

            esq = work.tile([NB, EDIM], F32, tag="esq")
            nc.vector.tensor_mul(esq[:], embsb, embsb)
            ensq = work.tile([NB, 1], F32, tag="ensq")
            nc.vector.tensor_reduce(ensq[:], esq[:], axis=AX.X, op=ALU.add)
            eln = work.tile([NB, 1], F32, tag="eln")
            nc.scalar.activation(out=eln[:], in_=ensq[:], func=AF.Ln)
            einv = work.tile([NB, 1], F32, tag="einv")
            nc.scalar.activation(out=einv[:], in_=eln[:], func=AF.Exp, scale=-0.5)
            embn = work.tile([NB, EDIM], F32, tag="embn")
            nc.vector.tensor_scalar_mul(out=embn[:], in0=embsb, scalar1=einv[:])
            embnT_ps = pspool.tile([EDIM, NB], F32, tag="small")
            nc.tensor.transpose(embnT_ps[:], embn[:], ident[0:NB, 0:NB])
            embnT = work.tile([EDIM, NB], F32, tag="embnT")
            nc.scalar.copy(out=embnT[:], in_=embnT_ps[:])

            simps = pspool.tile([BLOC, NB], F32, tag="small")
            nc.tensor.matmul(simps[:], rsb[:], embnT[:], start=True, stop=True)
            sim = work.tile([BLOC, NB], F32, tag="sim")
            nc.vector.tensor_scalar_mul(out=sim[:], in0=simps[:], scalar1=rinv[:])

            # ---------- softmax ----------
            mx = work.tile([BLOC, 1], F32, tag="mx")
            nc.vector.tensor_reduce(mx[:], sim[:], axis=AX.X, op=ALU.max)
            negmx = work.tile([BLOC, 1], F32, tag="negmx")
            nc.vector.tensor_scalar_mul(out=negmx[:], in0=mx[:], scalar1=-1.0)
            ex = work.tile([BLOC, NB], F32, tag="ex")
            s = work.tile([BLOC, 1], F32, tag="s")
            nc.scalar.activation(
                out=ex[:], in_=sim[:], func=AF.Exp, bias=negmx[:], scale=1.0,
                accum_out=s[:],
            )
            sinv = work.tile([BLOC, 1], F32, tag="sinv")
            nc.vector.reciprocal(sinv[:], s[:])
            wf = work.tile([BLOC, NB], F32, tag="wf")
            nc.vector.tensor_scalar_mul(out=wf[:], in0=ex[:], scalar1=sinv[:])

            # ---------- PE warmup: keep HAM clock ungated until w_eff ----------
            wl = bbt[:, BB_W1 : BB_W1 + 128]
            wr = bbt[:, BB_W1 : BB_W1 + 512]
            warm_ps = pswarm.tile([128, 512], F32, tag="warm")
            for _ in range(NWARM_PRE):
                nc.tensor.matmul(warm_ps[:], wl, wr, start=True, stop=True)

            # ---------- effective conv bias (both partition halves) ----------
            wfT_ps = pspool.tile([NB, BLOC], F32, tag="small")
            nc.tensor.transpose(wfT_ps[:], wf[:], ident[0:BLOC, 0:BLOC])
            wfT = work.tile([NB, BLOC], F32, tag="wfT")
            nc.scalar.copy(out=wfT[:], in_=wfT_ps[:])
            beff_ps = pspool.tile([128, BLOC], F32, tag="small")
            nc.tensor.matmul(
                beff_ps[0:64], cbsb, wfT[:], start=True, stop=True,
                tile_position=(0, 0),
            )
            nc.tensor.matmul(
                beff_ps[64:128], cbsb, wfT[:], start=True, stop=True,
                tile_position=(0, 64), skip_group_check=True,
            )
            beff2 = work.tile([128, BLOC], F32, tag="beff2")
            nc.scalar.copy(out=beff2[:], in_=beff_ps[:])

            # ---------- per-(sample, expert-pair) scale table ----------
            # wfsel[p, (b g)] = wf[b, 2g]   for p < 64
            #                 = wf[b, 2g+1] for p >= 64
            wfx_ps = pspool.tile([NB, 20], F32, tag="small")
            nc.tensor.matmul(wfx_ps[:], wf[:], e4sb, start=True, stop=True)
            wfx = work.tile([NB, 20], F32, tag="wfx")
            nc.scalar.copy(out=wfx[:], in_=wfx_ps[:])
            rhsE = work.tile([NB, 20], F32, tag="rhsE")
            nc.vector.tensor_mul(rhsE[:], wfx[:], maskE)
            rhsO = work.tile([NB, 20], F32, tag="rhsO")
            nc.vector.tensor_mul(rhsO[:], wfx[:], maskO)
            wfsel_ps = pspool.tile([128, 20], F32, tag="small")
            nc.tensor.matmul(
                wfsel_ps[0:64], ones10_64[:], rhsE[:], start=True, stop=True,
                tile_position=(0, 0), skip_group_check=True,
            )
            nc.tensor.matmul(
                wfsel_ps[64:128], ones10_64[:], rhsO[:], start=True, stop=True,
                tile_position=(0, 64), skip_group_check=True,
            )
            wfsel = work.tile([128, 20], F32, tag="wfsel")
            nc.scalar.copy(out=wfsel[:], in_=wfsel_ps[:])

            # scaled-identity stationaries: lhsw[b][g] = [wf[b,2g]*I; wf[b,2g+1]*I]
            lhsw = []
            for b in range(BLOC):
                row = []
                for g in range(NG):
                    lt = work.tile(
                        [128, 64], BF16, tag=f"lh{b}_{g}", name=f"lh{b}_{g}"
                    )
                    nc.vector.tensor_scalar_mul(
                        out=lt[:], in0=identPair,
                        scalar1=wfsel[:, 5 * b + g : 5 * b + g + 1],
                    )
                    row.append(lt)
                lhsw.append(row)

            for _ in range(NWARM_POST):
                nc.tensor.matmul(warm_ps[:], wl, wr, start=True, stop=True)
            warm_sink = work.tile([1, 1], F32, tag="warm_sink")
            nc.scalar.copy(out=warm_sink[:], in_=warm_ps[0:1, 0:1])

            # ---------- w_eff via PSUM-accumulated matmuls ----------
            # weff[0:64]  = weights for sample 2j   (PE row tile 0)
            # weff[64:128] = weights for sample 2j+1 (PE row tile 64)
            weffs = []
            for j in range(2):
                wpsA = psconv.tile([128, NFREE], F32, tag="psA")
                wpsB = psconv.tile([128, NFREE], F32, tag="psB")
                for g in range(NG):
                    st_, sp = (g == 0), (g == NG - 1)
                    for half, b in ((0, 2 * j), (1, 2 * j + 1)):
                        lo, hi = 64 * half, 64 * half + 64
                        nc.tensor.matmul(
                            wpsA[lo:hi, 0:FDH], lhsw[b][g], cwt[:, g, 0:FDH],
                            start=st_, stop=sp, tile_position=(0, 64 * half),
                            skip_group_check=True,
                        )
                        nc.tensor.matmul(
                            wpsB[lo:hi, 0:FDH], lhsw[b][g], cwt[:, g, FDH:FD],
                            start=st_, stop=sp, tile_position=(0, 64 * half),
                            skip_group_check=True,
                        )
                weff = work.tile(
                    [128, NTAP, COUT], BF16, tag=f"weff{j}", name=f"weff{j}"
                )
                wv = weff[:].rearrange("p t c -> p (t c)")
                nc.scalar.copy(out=wv[:, 0:FDH], in_=wpsA[:, 0:FDH])
                nc.scalar.copy(out=wv[:, FDH:FD], in_=wpsB[:, 0:FDH])
                weffs.append(weff)

            # ---------- conv: quad-tiled 9-tap shifted matmuls ----------
            stgs = [None, None]
            for j in range(2):
                weff = weffs[j]
                for g in range(NGRP):
                    h_e = 2 * g * GR       # even chunk start row
                    h_o = h_e + GR         # odd chunk start row
                    psA = psconv.tile([128, NFREE], F32, tag="psA")
                    psB = psconv.tile([128, NFREE], F32, tag="psB")
                    for t in range(NTAP):
                        oe = h_e * W + TAP_OFF[t]
                        oo = h_o * W + TAP_OFF[t]
                        st_, sp = (t == 0), (t == NTAP - 1)
                        nc.tensor.matmul(
                            psA[0:64], weff[0:64, t, :], xt[j][0:64, oe : oe + NFREE],
                            start=st_, stop=sp, tile_position=(0, 0),
                            skip_group_check=True,
                        )
                        nc.tensor.matmul(
                            psA[64:128], weff[0:64, t, :], xt[j][0:64, oo : oo + NFREE],
                            start=st_, stop=sp, tile_position=(0, 64),
                            skip_group_check=True,
                        )
                        nc.tensor.matmul(
                            psB[0:64], weff[64:128, t, :], xt[j][64:128, oe : oe + NFREE],
                            start=st_, stop=sp, tile_position=(64, 0),
                            skip_group_check=True,
                        )
                        nc.tensor.matmul(
                            psB[64:128], weff[64:128, t, :], xt[j][64:128, oo : oo + NFREE],
                            start=st_, stop=sp, tile_position=(64, 64),
                            skip_group_check=True,
                        )
                    # evacuate both parities in one op per sample (scalar),
                    # +bias, trim 58->56 cols
                    gi = g % 2
                    for half, ps in ((0, psA), (1, psB)):
                        b = 2 * j + half
                        if gi == 0:
                            stgs[half] = stage.tile(
                                [128, 2, GR, OW], F32,
                                tag=f"stg{half}", name=f"stg{half}",
                            )
                        st = stgs[half]
                        pv = ps[:].rearrange("p (r w) -> p r w", w=W)[:, :, 0:OW]
                        nc.scalar.activation(
                            out=st[:, gi], in_=pv, func=AF.Identity,
                            bias=beff2[:, b : b + 1], scale=1.0,
                        )
                        # after 2 groups: store both row-parities of the window
                        if gi == 1:
                            dv = out[b].rearrange(
                                "c (G g2 r) w -> g2 c G r w", G=NGRP, g2=2
                            )
                            nc.sync.dma_start(
                                out=dv[0, :, g - 1 : g + 1], in_=st[0:64]
                            )
                            nc.sync.dma_start(
                                out=dv[1, :, g - 1 : g + 1], in_=st[64:128]
                            )

    fix_sync_waits(nc)
    return nc


_NC = None


def _get_nc():
    global _NC
    if _NC is None:
        _NC = build()
    return _NC


def make_in_maps(inputs):
    bf16 = ml_dtypes.bfloat16
    x = np.asarray(inputs["x"], dtype=np.float32).reshape(32, CIN, HW)
    rvec = np.asarray(inputs["routing_vector"], dtype=np.float32)
    W1 = np.asarray(inputs["W1"], dtype=np.float32)
    b1 = np.asarray(inputs["b1"], dtype=np.float32)
    W2 = np.asarray(inputs["W2"], dtype=np.float32)
    b2 = np.asarray(inputs["b2"], dtype=np.float32)
    emb = np.asarray(inputs["emb"], dtype=np.float32)
    conv_w = np.asarray(inputs["conv_w"], dtype=np.float32)
    conv_b = np.asarray(inputs["conv_b"], dtype=np.float32)

    # conv_w[n, co, ci, ky, kx] -> cwp[(n%2)*64+ci, n//2, (ky kx)*co], bf16
    cwpt = conv_w.transpose(2, 0, 3, 4, 1).reshape(CIN, NB, FD)   # [ci, n, tc]
    cwpa = np.zeros((128, NG, FD), np.float32)
    for g in range(NG):
        cwpa[0:64, g] = cwpt[:, 2 * g]
        cwpa[64:128, g] = cwpt[:, 2 * g + 1]
    cwpa = np.ascontiguousarray(cwpa).astype(bf16)

    # blkb (bf16): per-core rvT + routing weights + stacked identity pair
    w1blk = W1.reshape(4, 128, HID).transpose(1, 0, 2).reshape(128, 512)
    blkb_shared = np.zeros((128, BB_D), np.float32)
    blkb_shared[:, BB_W1 : BB_W1 + 512] = w1blk
    blkb_shared[:, BB_W2 : BB_W2 + 64] = W2
    eye64 = np.eye(64, dtype=np.float32)
    blkb_shared[0:64, BB_IP : BB_IP + 64] = eye64
    blkb_shared[64:128, BB_IP : BB_IP + 64] = eye64

    # blk2 (fp32): emb, conv bias, identity, wfsel helper constants, biases
    blk2a = np.zeros((128, B2_D), np.float32)
    blk2a[0:NB, B2_EMB : B2_EMB + 64] = emb
    blk2a[0:NB, B2_CB : B2_CB + 64] = conv_b
    blk2a[0:16, B2_ID : B2_ID + 16] = np.eye(16, dtype=np.float32)
    for b in range(BLOC):
        for g in range(NG):
            blk2a[b, B2_E4 + 5 * b + g] = 1.0
            blk2a[2 * g, B2_ME + 5 * b + g] = 1.0
            blk2a[2 * g + 1, B2_MO + 5 * b + g] = 1.0
    blk2a[:, B2_B1] = b1
    blk2a[0:EDIM, B2_B2] = b2

    xb = x.astype(bf16)
    in_maps = []
    for c in range(NCORES):
        blkba = blkb_shared.copy()
        rvc = rvec[BLOC * c : BLOC * (c + 1)]          # [4, 512]
        # rvT[p, c, b] = rv[b, 128c + p]
        rvt = rvc.T.reshape(4, 128, BLOC).transpose(1, 0, 2).reshape(128, 16)
        blkba[:, BB_RVT : BB_RVT + 16] = rvt
        in_maps.append(
            {
                "blkb": blkba.astype(bf16),
                "blk2": blk2a,
                "cwp": cwpa,
                "x": np.ascontiguousarray(xb[BLOC * c : BLOC * (c + 1)]),
            }
        )
    return in_maps


def kernel(**inputs):
    from concourse.bass_utils import run_bass_kernel_spmd

    nc = _get_nc()
    in_maps = make_in_maps(inputs)
    res = run_bass_kernel_spmd(nc, in_maps, core_ids=list(range(NCORES)))
    return np.concatenate([r["out"] for r in res.results], axis=0)


# revision 10
# speedup vs baseline: 2.9822x; 1.0168x over previous
"""MoE routing layer on 8 Trainium2 NeuronCores (data-parallel over batch).

Per core (4 samples):
  routing MLP -> cosine sim vs embeddings -> softmax weights wf[4,10]
  w_eff[b] = sum_n wf[b,n] * conv_w[n]  (conv is linear in weights ->
  10x fewer conv FLOPs than materializing all expert convs)
  out[b] = conv2d(x[b], w_eff[b]) + b_eff[b]

Conv path runs in bf16 (x, w_eff) accumulating fp32 in PSUM. The conv is
9 shifted matmuls over the flat 58-wide grid; the PE array is quad-tiled
(2 samples on row halves x 2 chunk parities on column halves) so all four
64x64 quadrants stream concurrently.

w_eff is ALSO built on the PE: conv weights are host-packed as expert
pairs on partition halves (cwp[(n2 ci), g, tap*cout]), and for each
expert-pair g a [128,64] stationary operand holding two stacked scaled
identities (wf[b,2g]*I ; wf[b,2g+1]*I) contracts against it, accumulating
the 10-expert weighted sum directly in PSUM. The DVE only builds the
small scaled identities — the [128,576]-sized elementwise work that made
vector/gpsimd the bottleneck is gone.
"""
import sys

sys.path.insert(0, "/opt/trn_rl_repo")

import numpy as np
import ml_dtypes

import concourse.bass as bass
import concourse.mybir as mybir
from concourse.tile import TileContext

F32 = mybir.dt.float32
BF16 = mybir.dt.bfloat16
AF = mybir.ActivationFunctionType
ALU = mybir.AluOpType
AX = mybir.AxisListType

NCORES = 8
BLOC = 4           # samples per core
CIN = 64
COUT = 64
H = W = 58
HW = H * W         # 3364
OH = OW = 56
NB = 10            # experts
NG = 5             # expert pairs
EDIM = 64
RSIZE = 512
HID = 128
NTAP = 9
FD = NTAP * COUT   # 576
FDH = FD // 2      # 288 (PSUM bank-sized half)
GR = 7             # output rows per chunk
NGRP = 4           # 4 groups x (even chunk + odd chunk) x 7 rows = 56
NFREE = GR * W     # 406 <= 512 (one PSUM bank)
TAP_OFF = [dy * W + dx for dy in range(3) for dx in range(3)]
NWARM_PRE = 10     # PE warmup matmuls (HAM clock-gate) after routing
NWARM_POST = 4     # bridge the PE gap from routing to the w_eff matmuls

# blkb column layout (128 partitions, bf16): routing weights + identities
BB_RVT = 0                  # [128, 4c, 4b]
BB_W1 = BB_RVT + 16         # [128, 4c, 128m]
BB_W2 = BB_W1 + 512         # [128, 64]
BB_IP = BB_W2 + 64          # [128, 64] two stacked 64x64 identities
BB_D = BB_IP + 64           # 656

# blk2 column layout (128 partitions, fp32)
B2_EMB = 0                  # [10, 64]
B2_CB = B2_EMB + 64         # [10, 64]
B2_ID = B2_CB + 64          # [16, 16]
B2_E4 = B2_ID + 16          # [4, 20]  E4[b',(b g)] = delta(b,b')
B2_ME = B2_E4 + 20          # [10, 20] mask_even[n,(b g)] = delta(n,2g)
B2_MO = B2_ME + 20          # [10, 20] mask_odd[n,(b g)]  = delta(n,2g+1)
B2_B1 = B2_MO + 20          # [128, 1]
B2_B2 = B2_B1 + 1           # [64, 1]
B2_D = B2_B2 + 1            # 206


def fix_sync_waits(nc, cap=2):
    """This walrus build allows at most `cap` sem waits per instruction.
    Splice same-engine NoOps carrying the excess waits right before any
    over-subscribed instruction (waits happen earlier => same semantics)."""
    uid = [0]
    for f in nc.m.functions:
        for blk in f.blocks:
            insts = blk.instructions  # live list
            i = 0
            while i < len(insts):
                inst = insts[i]
                si = inst.sync_info
                waits = list(si.on_wait) if si and si.on_wait else []
                icap = 1
                if len(waits) <= icap:
                    i += 1
                    continue
                keep, excess = waits[-icap:], waits[:-icap]
                for k in range(0, len(excess), icap):
                    nop = mybir.InstNoOp(
                        name=f"{inst.name}-wsplit{uid[0]}", ins=[], outs=[]
                    )
                    uid[0] += 1
                    nop.engine = inst.engine
                    nop.sync_info = mybir.SyncInfo(
                        on_wait=excess[k : k + icap], on_update=[]
                    )
                    nc.register_instruction(nop, overwrite=True)
                    insts.insert(i, nop)
                    i += 1
                inst.sync_info = mybir.SyncInfo(
                    on_wait=keep,
                    on_update=list(si.on_update) if si and si.on_update else [],
                )
                i += 1


def build():
    nc = bass.Bass(num_swdge_queues=4)
    blkb = nc.dram_tensor("blkb", [128, BB_D], BF16, kind="ExternalInput")
    blk2 = nc.dram_tensor("blk2", [128, B2_D], F32, kind="ExternalInput")
    cwp = nc.dram_tensor("cwp", [128, NG, FD], BF16, kind="ExternalInput")
    x = nc.dram_tensor("x", [BLOC, CIN, HW], BF16, kind="ExternalInput")
    out = nc.dram_tensor("out", [BLOC, COUT, OH, OW], F32, kind="ExternalOutput")

    with TileContext(nc) as tc:
        with (
            tc.tile_pool(name="consts", bufs=1) as consts,
            tc.tile_pool(name="work", bufs=2) as work,
            tc.tile_pool(name="stage", bufs=3) as stage,
            tc.tile_pool(name="ps", bufs=2, space="PSUM") as pspool,
            tc.tile_pool(name="psconv", bufs=2, space="PSUM") as psconv,
            tc.tile_pool(name="pswarm", bufs=1, space="PSUM") as pswarm,
        ):
            # ---------- DMA in (all on sync hw queue, critical first) ----------
            bbt = consts.tile([128, BB_D], BF16, tag="bbt")
            nc.sync.dma_start(out=bbt[:], in_=blkb[:])
            b2t = consts.tile([128, B2_D], F32, tag="b2t")
            nc.sync.dma_start(out=b2t[:], in_=blk2[:])
            cwt = consts.tile([128, NG, FD], BF16, tag="cwt")
            nc.sync.dma_start(out=cwt[:, 0:3], in_=cwp[:, 0:3])
            nc.sync.dma_start(out=cwt[:, 3:NG], in_=cwp[:, 3:NG])

            xt = []
            for j in range(2):
                t = consts.tile([128, HW + 4], BF16, tag=f"xt{j}")
                nc.vector.memset(t[:, HW : HW + 4], 0.0)
                xt.append(t)
            for b in range(BLOC):
                j, half = divmod(b, 2)
                nc.sync.dma_start(
                    out=xt[j][64 * half : 64 * half + 64, 0:HW], in_=x[b]
                )

            ones64 = consts.tile([EDIM, 1], F32, tag="ones64")
            nc.vector.memset(ones64[:], 1.0)
            ones10_64 = consts.tile([NB, 64], F32, tag="ones10_64")
            nc.vector.memset(ones10_64[:], 1.0)

            # views into the packed blocks
            rvT = bbt[:, BB_RVT : BB_RVT + 16].rearrange("p (c b) -> p c b", c=4)
            w1sb = bbt[:, BB_W1 : BB_W1 + 512].rearrange("p (c m) -> p c m", c=4)
            w2sb = bbt[:, BB_W2 : BB_W2 + 64]
            identPair = bbt[:, BB_IP : BB_IP + 64]
            embsb = b2t[0:NB, B2_EMB : B2_EMB + 64]
            cbsb = b2t[0:NB, B2_CB : B2_CB + 64]
            ident = b2t[0:16, B2_ID : B2_ID + 16]
            e4sb = b2t[0:4, B2_E4 : B2_E4 + 20]
            maskE = b2t[0:NB, B2_ME : B2_ME + 20]
            maskO = b2t[0:NB, B2_MO : B2_MO + 20]
            b1sb = b2t[:, B2_B1 : B2_B1 + 1]
            b2sb = b2t[0:EDIM, B2_B2 : B2_B2 + 1]

            # ---------- routing MLP (bf16 matmuls; rv pre-transposed) ----------
            h1 = pspool.tile([HID, BLOC], F32, tag="small")
            for c in range(4):
                nc.tensor.matmul(
                    h1[:], w1sb[:, c, :], rvT[:, c, :], start=(c == 0), stop=(c == 3)
                )
            h1r = work.tile([HID, BLOC], BF16, tag="h1r")
            nc.scalar.activation(
                out=h1r[:], in_=h1[:], func=AF.Relu, bias=b1sb, scale=1.0
            )
            rps = pspool.tile([EDIM, BLOC], F32, tag="small")
            nc.tensor.matmul(rps[:], w2sb, h1r[:], start=True, stop=True)
            rsb = work.tile([EDIM, BLOC], F32, tag="rsb")
            nc.scalar.activation(
                out=rsb[:], in_=rps[:], func=AF.Identity, bias=b2sb, scale=1.0
            )

            # ---------- cosine similarity ----------
            rsq = work.tile([EDIM, BLOC], F32, tag="rsq")
            nc.vector.tensor_mul(rsq[:], rsb[:], rsb[:])
            nsq = pspool.tile([BLOC, 1], F32, tag="small")
            nc.tensor.matmul(nsq[:], rsq[:], ones64[:], start=True, stop=True)
            rln = work.tile([BLOC, 1], F32, tag="rln")
            nc.scalar.activation(out=rln[:], in_=nsq[:], func=AF.Ln)
            rinv = work.tile([BLOC, 1], F32, tag="rinv")
            nc.scalar.activation(out=rinv[:], in_=rln[:], func=AF.Exp, scale=-0.5)

            esq = work.tile([NB, EDIM], F32, tag="esq")
            nc.vector.tensor_mul(esq[:], embsb, embsb)
            ensq = work.tile([NB, 1], F32, tag="ensq")
            nc.vector.tensor_reduce(ensq[:], esq[:], axis=AX.X, op=ALU.add)
            eln = work.tile([NB, 1], F32, tag="eln")
            nc.scalar.activation(out=eln[:], in_=ensq[:], func=AF.Ln)
            einv = work.tile([NB, 1], F32, tag="einv")
            nc.scalar.activation(out=einv[:], in_=eln[:], func=AF.Exp, scale=-0.5)
            embn = work.tile([NB, EDIM], F32, tag="embn")
            nc.vector.tensor_scalar_mul(out=embn[:], in0=embsb, scalar1=einv[:])
            embnT_ps = pspool.tile([EDIM, NB], F32, tag="small")
            nc.tensor.transpose(embnT_ps[:], embn[:], ident[0:NB, 0:NB])
            embnT = work.tile([EDIM, NB], F32, tag="embnT")
            nc.scalar.copy(out=embnT[:], in_=embnT_ps[:])

            simps = pspool.tile([BLOC, NB], F32, tag="small")
            nc.tensor.matmul(simps[:], rsb[:], embnT[:], start=True, stop=True)
            sim = work.tile([BLOC, NB], F32, tag="sim")
            nc.vector.tensor_scalar_mul(out=sim[:], in0=simps[:], scalar1=rinv[:])

            # ---------- softmax ----------
            mx = work.tile([BLOC, 1], F32, tag="mx")
            nc.vector.tensor_reduce(mx[:], sim[:], axis=AX.X, op=ALU.max)
            negmx = work.tile([BLOC, 1], F32, tag="negmx")
            nc.vector.tensor_scalar_mul(out=negmx[:], in0=mx[:], scalar1=-1.0)
            ex = work.tile([BLOC, NB], F32, tag="ex")
            s = work.tile([BLOC, 1], F32, tag="s")
            nc.scalar.activation(
                out=ex[:], in_=sim[:], func=AF.Exp, bias=negmx[:], scale=1.0,
                accum_out=s[:],
            )
            sinv = work.tile([BLOC, 1], F32, tag="sinv")
            nc.vector.reciprocal(sinv[:], s[:])
            wf = work.tile([BLOC, NB], F32, tag="wf")
            nc.vector.tensor_scalar_mul(out=wf[:], in0=ex[:], scalar1=sinv[:])

            # ---------- PE warmup: keep HAM clock ungated until w_eff ----------
            wl = bbt[:, BB_W1 : BB_W1 + 128]
            wr = bbt[:, BB_W1 : BB_W1 + 512]
            warm_ps = pswarm.tile([128, 512], F32, tag="warm")
            for _ in range(NWARM_PRE):
                nc.tensor.matmul(warm_ps[:], wl, wr, start=True, stop=True)

            # ---------- effective conv bias (both partition halves) ----------
            wfT_ps = pspool.tile([NB, BLOC], F32, tag="small")
            nc.tensor.transpose(wfT_ps[:], wf[:], ident[0:BLOC, 0:BLOC])
            wfT = work.tile([NB, BLOC], F32, tag="wfT")
            nc.scalar.copy(out=wfT[:], in_=wfT_ps[:])
            beff_ps = pspool.tile([128, BLOC], F32, tag="small")
            nc.tensor.matmul(
                beff_ps[0:64], cbsb, wfT[:], start=True, stop=True,
                tile_position=(0, 0),
            )
            nc.tensor.matmul(
                beff_ps[64:128], cbsb, wfT[:], start=True, stop=True,
                tile_position=(0, 64), skip_group_check=True,
            )
            beff2 = work.tile([128, BLOC], F32, tag="beff2")
            nc.scalar.copy(out=beff2[:], in_=beff_ps[:])

            # ---------- per-(sample, expert-pair) scale table ----------
            # wfsel[p, (b g)] = wf[b, 2g]   for p < 64
            #                 = wf[b, 2g+1] for p >= 64
            wfx_ps = pspool.tile([NB, 20], F32, tag="small")
            nc.tensor.matmul(wfx_ps[:], wf[:], e4sb, start=True, stop=True)
            wfx = work.tile([NB, 20], F32, tag="wfx")
            nc.scalar.copy(out=wfx[:], in_=wfx_ps[:])
            rhsE = work.tile([NB, 20], F32, tag="rhsE")
            nc.vector.tensor_mul(rhsE[:], wfx[:], maskE)
            rhsO = work.tile([NB, 20], F32, tag="rhsO")
            nc.vector.tensor_mul(rhsO[:], wfx[:], maskO)
            wfsel_ps = pspool.tile([128, 20], F32, tag="small")
            nc.tensor.matmul(
                wfsel_ps[0:64], ones10_64[:], rhsE[:], start=True, stop=True,
                tile_position=(0, 0), skip_group_check=True,
            )
            nc.tensor.matmul(
                wfsel_ps[64:128], ones10_64[:], rhsO[:], start=True, stop=True,
                tile_position=(0, 64), skip_group_check=True,
            )
            wfsel = work.tile([128, 20], F32, tag="wfsel")
            nc.scalar.copy(out=wfsel[:], in_=wfsel_ps[:])

            # scaled-identity stationaries: lhsw[b][g] = [wf[b,2g]*I; wf[b,2g+1]*I]
            lhsw = []
            for b in range(BLOC):
                row = []
                for g in range(NG):
                    lt = work.tile(
                        [128, 64], BF16, tag=f"lh{b}_{g}", name=f"lh{b}_{g}"
                    )
                    nc.vector.tensor_scalar_mul(
                        out=lt[:], in0=identPair,
                        scalar1=wfsel[:, 5 * b + g : 5 * b + g + 1],
                    )
                    row.append(lt)
                lhsw.append(row)

            for _ in range(NWARM_POST):
                nc.tensor.matmul(warm_ps[:], wl, wr, start=True, stop=True)
            warm_sink = work.tile([1, 1], F32, tag="warm_sink")
            nc.scalar.copy(out=warm_sink[:], in_=warm_ps[0:1, 0:1])

            # ---------- w_eff via PSUM-accumulated matmuls ----------
            # weff[0:64]  = weights for sample 2j   (PE row tile 0)
            # weff[64:128] = weights for sample 2j+1 (PE row tile 64)
            weffs = []
            for j in range(2):
                wpsA = psconv.tile([128, NFREE], F32, tag="psA")
                wpsB = psconv.tile([128, NFREE], F32, tag="psB")
                for g in range(NG):
                    st_, sp = (g == 0), (g == NG - 1)
                    for half, b in ((0, 2 * j), (1, 2 * j + 1)):
                        lo, hi = 64 * half, 64 * half + 64
                        nc.tensor.matmul(
                            wpsA[lo:hi, 0:FDH], lhsw[b][g], cwt[:, g, 0:FDH],
                            start=st_, stop=sp, tile_position=(0, 64 * half),
                            skip_group_check=True,
                        )
                        nc.tensor.matmul(
                            wpsB[lo:hi, 0:FDH], lhsw[b][g], cwt[:, g, FDH:FD],
                            start=st_, stop=sp, tile_position=(0, 64 * half),
                            skip_group_check=True,
                        )
                weff = work.tile(
                    [128, NTAP, COUT], BF16, tag=f"weff{j}", name=f"weff{j}"
                )
                wv = weff[:].rearrange("p t c -> p (t c)")
                nc.scalar.copy(out=wv[:, 0:FDH], in_=wpsA[:, 0:FDH])
                nc.scalar.copy(out=wv[:, FDH:FD], in_=wpsB[:, 0:FDH])
                weffs.append(weff)

            # ---------- conv: quad-tiled 9-tap shifted matmuls ----------
            stgs = [None, None]
            for j in range(2):
                weff = weffs[j]
                for g in range(NGRP):
                    h_e = 2 * g * GR       # even chunk start row
                    h_o = h_e + GR         # odd chunk start row
                    psA = psconv.tile([128, NFREE], F32, tag="psA")
                    psB = psconv.tile([128, NFREE], F32, tag="psB")
                    for t in range(NTAP):
                        oe = h_e * W + TAP_OFF[t]
                        oo = h_o * W + TAP_OFF[t]
                        st_, sp = (t == 0), (t == NTAP - 1)
                        nc.tensor.matmul(
                            psA[0:64], weff[0:64, t, :], xt[j][0:64, oe : oe + NFREE],
                            start=st_, stop=sp, tile_position=(0, 0),
                            skip_group_check=True,
                        )
                        nc.tensor.matmul(
                            psA[64:128], weff[0:64, t, :], xt[j][0:64, oo : oo + NFREE],
                            start=st_, stop=sp, tile_position=(0, 64),
                            skip_group_check=True,
                        )
                        nc.tensor.matmul(
                            psB[0:64], weff[64:128, t, :], xt[j][64:128, oe : oe + NFREE],
                            start=st_, stop=sp, tile_position=(64, 0),
                            skip_group_check=True,
                        )
                        nc.tensor.matmul(
                            psB[64:128], weff[64:128, t, :], xt[j][64:128, oo : oo + NFREE],
                            start=st_, stop=sp, tile_position=(64, 64),
                            skip_group_check=True,
                        )
                    # evacuate both parities in one op per sample (scalar),
                    # +bias, trim 58->56 cols
                    gi = g % 2
                    for half, ps in ((0, psA), (1, psB)):
                        b = 2 * j + half
                        if gi == 0:
                            stgs[half] = stage.tile(
                                [128, 2, GR, OW], F32,
                                tag=f"stg{half}", name=f"stg{half}",
                            )
                        st = stgs[half]
                        pv = ps[:].rearrange("p (r w) -> p r w", w=W)[:, :, 0:OW]
                        if half == 0:
                            nc.scalar.activation(
                                out=st[:, gi], in_=pv, func=AF.Identity,
                                bias=beff2[:, b : b + 1], scale=1.0,
                            )
                        else:
                            nc.vector.tensor_scalar_add(
                                out=st[:, gi], in0=pv,
                                scalar1=beff2[:, b : b + 1],
                            )
                        # after 2 groups: store both row-parities of the window
                        if gi == 1:
                            dv = out[b].rearrange(
                                "c (G g2 r) w -> g2 c G r w", G=NGRP, g2=2
                            )
                            nc.sync.dma_start(
                                out=dv[0, :, g - 1 : g + 1], in_=st[0:64]
                            )
                            nc.sync.dma_start(
                                out=dv[1, :, g - 1 : g + 1], in_=st[64:128]
                            )

    fix_sync_waits(nc)
    return nc


_NC = None


def _get_nc():
    global _NC
    if _NC is None:
        _NC = build()
    return _NC


def make_in_maps(inputs):
    bf16 = ml_dtypes.bfloat16
    x = np.asarray(inputs["x"], dtype=np.float32).reshape(32, CIN, HW)
    rvec = np.asarray(inputs["routing_vector"], dtype=np.float32)
    W1 = np.asarray(inputs["W1"], dtype=np.float32)
    b1 = np.asarray(inputs["b1"], dtype=np.float32)
    W2 = np.asarray(inputs["W2"], dtype=np.float32)
    b2 = np.asarray(inputs["b2"], dtype=np.float32)
    emb = np.asarray(inputs["emb"], dtype=np.float32)
    conv_w = np.asarray(inputs["conv_w"], dtype=np.float32)
    conv_b = np.asarray(inputs["conv_b"], dtype=np.float32)

    # conv_w[n, co, ci, ky, kx] -> cwp[(n%2)*64+ci, n//2, (ky kx)*co], bf16
    cwpt = conv_w.transpose(2, 0, 3, 4, 1).reshape(CIN, NB, FD)   # [ci, n, tc]
    cwpa = np.zeros((128, NG, FD), np.float32)
    for g in range(NG):
        cwpa[0:64, g] = cwpt[:, 2 * g]
        cwpa[64:128, g] = cwpt[:, 2 * g + 1]
    cwpa = np.ascontiguousarray(cwpa).astype(bf16)

    # blkb (bf16): per-core rvT + routing weights + stacked identity pair
    w1blk = W1.reshape(4, 128, HID).transpose(1, 0, 2).reshape(128, 512)
    blkb_shared = np.zeros((128, BB_D), np.float32)
    blkb_shared[:, BB_W1 : BB_W1 + 512] = w1blk
    blkb_shared[:, BB_W2 : BB_W2 + 64] = W2
    eye64 = np.eye(64, dtype=np.float32)
    blkb_shared[0:64, BB_IP : BB_IP + 64] = eye64
    blkb_shared[64:128, BB_IP : BB_IP + 64] = eye64

    # blk2 (fp32): emb, conv bias, identity, wfsel helper constants, biases
    blk2a = np.zeros((128, B2_D), np.float32)
    blk2a[0:NB, B2_EMB : B2_EMB + 64] = emb
    blk2a[0:NB, B2_CB : B2_CB + 64] = conv_b
    blk2a[0:16, B2_ID : B2_ID + 16] = np.eye(16, dtype=np.float32)
    for b in range(BLOC):
        for g in range(NG):
            blk2a[b, B2_E4 + 5 * b + g] = 1.0
            blk2a[2 * g, B2_ME + 5 * b + g] = 1.0
            blk2a[2 * g + 1, B2_MO + 5 * b + g] = 1.0
    blk2a[:, B2_B1] = b1
    blk2a[0:EDIM, B2_B2] = b2

    xb = x.astype(bf16)
    in_maps = []
    for c in range(NCORES):
        blkba = blkb_shared.copy()
        rvc = rvec[BLOC * c : BLOC * (c + 1)]          # [4, 512]
        # rvT[p, c, b] = rv[b, 128c + p]
        rvt = rvc.T.reshape(4, 128, BLOC).transpose(1, 0, 2).reshape(128, 16)
        blkba[:, BB_RVT : BB_RVT + 16] = rvt
        in_maps.append(
            {
                "blkb": blkba.astype(bf16),
                "blk2": blk2a,
                "cwp": cwpa,
                "x": np.ascontiguousarray(xb[BLOC * c : BLOC * (c + 1)]),
            }
        )
    return in_maps


def kernel(**inputs):
    from concourse.bass_utils import run_bass_kernel_spmd

    nc = _get_nc()
    in_maps = make_in_maps(inputs)
    res = run_bass_kernel_spmd(nc, in_maps, core_ids=list(range(NCORES)))
    return np.concatenate([r["out"] for r in res.results], axis=0)
